# revision 1
# baseline (speedup 1.0000x reference)
"""LocalGOCor (PWC-Net local correlation, radius 4) on 8 Trainium2 NeuronCores.

scores[b, d, y, x] = sum_c (gain * f[b,c,y,x]) * q_zeropad[b, c, y+dy, x+dx]
for d = dy*9+dx, dy/dx in [0,9)  (displacement dy-4, dx-4).

Strategy (data-parallel over batch, 2 samples per core):
  - Image tiled into 8y x 16x pixel blocks (M=128 = PSUM partitions).
  - Per block one TensorE matmul: lhsT = F[c, block 128 pixels] (K=C=128),
    rhs = padded-Q window [c, 16y x 24x] (N=384, one PSUM bank).
    PSUM[p=(ys,xs), n=(wy,wx)] holds all pairwise dots; the 81 useful
    displacement values per pixel live at (wy,wx) = (ys+dy, xs+dx).
  - Matmul operands must be single-free-dim APs, so F is packed
    block-contiguously on the host, and Q is restaged on-chip into eight
    24-column panels (pitch 24 -> any 16-row window is contiguous).
  - ACT/DVE copy PSUM->SBUF with scale=init_gain, downcast to bf16.
  - Dense bf16 block tensor DMA'd out; the band ("diagonal") extraction is a
    zero-copy numpy as_strided shear on the host during unsharding
    (device-side extraction would be DMA-descriptor-bound).
"""

import os
import numpy as np

B, C, H, W = 16, 128, 128, 128
R = 4
ND = 2 * R + 1            # 9 displacements per axis
NCORES = 8
BLOC = B // NCORES        # 2 samples per core
BY, BX = 8, 16            # pixels per block -> M = 128
WY, WX = BY + 2 * R, BX + 2 * R   # 16, 24 query window
NWIN = WY * WX            # 384 (<= 512: one PSUM bank)
YBLK = 32                 # image rows per chunk
NYB = H // YBLK           # 4
QROWS = YBLK + 2 * R      # 40
NPAN = W // BX            # 8 x-panels
GW = 4                    # blocks per PSUM group (4 banks)
NG = (YBLK // BY) * NPAN // GW    # 8 groups per (b, yblk)
XH = NPAN // GW           # 2 x-halves
NBLK = NG * GW            # 32 blocks per (b, yblk)

_CACHE = {}
LAST_RESULTS = None


def _build(mm_dtype: str):
    import concourse.bacc as bacc
    import concourse.tile as tile
    import concourse.mybir as mybir
    from contextlib import ExitStack

    nc = bacc.Bacc(
        "TRN2",
        target_bir_lowering=False,
        debug=False,
        enable_asserts=False,
        num_devices=NCORES,
    )
    f32 = mybir.dt.float32
    bf16 = mybir.dt.bfloat16
    mmdt = {"float32": f32, "float32r": mybir.dt.float32r,
            "bfloat16": bf16}[mm_dtype]

    # f is host-packed: [BLOC, C, NYB, NBLK, BY*BX] block-contiguous pixels
    f_dram = nc.dram_tensor("f", [BLOC, C, NYB, NBLK, BY * BX], f32,
                            kind="ExternalInput").ap()
    q_dram = nc.dram_tensor("q", [BLOC, C, H, W], f32, kind="ExternalInput").ap()
    g_dram = nc.dram_tensor("gain", [C, 1], f32, kind="ExternalInput").ap()
    o_dram = nc.dram_tensor(
        "out", [BLOC, NYB, NG, C, GW, NWIN], bf16, kind="ExternalOutput"
    ).ap()

    with tile.TileContext(nc) as tc, ExitStack() as ctx:
        qlpool = ctx.enter_context(tc.tile_pool(name="qlpool", bufs=2))
        qxpool = ctx.enter_context(tc.tile_pool(name="qxpool", bufs=2))
        fpool = ctx.enter_context(tc.tile_pool(name="fpool", bufs=2))
        opool = ctx.enter_context(tc.tile_pool(name="opool", bufs=4))
        gpool = ctx.enter_context(tc.tile_pool(name="gpool", bufs=1))
        pspool = ctx.enter_context(tc.tile_pool(name="pspool", bufs=2, space="PSUM"))

        gain_sb = gpool.tile([C, 1], f32)
        nc.sync.dma_start(out=gain_sb[:, :], in_=g_dram[:, :])

        for b in range(BLOC):
            for yb in range(NYB):
                ql = qlpool.tile([C, QROWS, W], f32, tag="ql")
                qx = qxpool.tile([C, NPAN, QROWS, WX], f32, tag="qx")
                ft = fpool.tile([C, NBLK, BY * BX], f32, tag="ft")

                nc.sync.dma_start(out=ft[:, :, :], in_=f_dram[b, :, yb, :, :])

                # ql row r <-> padded row yb*YBLK + r <-> real row yb*YBLK+r-4
                r_lo = yb * YBLK - R
                r_hi = yb * YBLK + YBLK + R
                lo_clip, hi_clip = max(r_lo, 0), min(r_hi, H)
                t_lo = lo_clip - r_lo
                t_hi = t_lo + (hi_clip - lo_clip)
                if t_lo > 0:
                    nc.vector.memset(ql[:, 0:t_lo, :], 0.0)
                if t_hi < QROWS:
                    nc.vector.memset(ql[:, t_hi:QROWS, :], 0.0)
                nc.sync.dma_start(
                    out=ql[:, t_lo:t_hi, :],
                    in_=q_dram[b, :, lo_clip:hi_clip, :],
                )

                # build 24-wide panels; panel j covers padded cols
                # [16j, 16j+24) = real cols [16j-4, 16j+20)
                for j in range(NPAN):
                    c_lo = 16 * j - R
                    c_hi = c_lo + WX
                    cl, ch = max(c_lo, 0), min(c_hi, W)
                    p_lo = cl - c_lo
                    p_hi = p_lo + (ch - cl)
                    if p_lo > 0:
                        nc.vector.memset(qx[:, j, :, 0:p_lo], 0.0)
                    if p_hi < WX:
                        nc.vector.memset(qx[:, j, :, p_hi:WX], 0.0)
                    if j % 2 == 0:
                        nc.vector.tensor_copy(qx[:, j, :, p_lo:p_hi],
                                              ql[:, :, cl:ch])
                    else:
                        nc.scalar.copy(qx[:, j, :, p_lo:p_hi], ql[:, :, cl:ch])

                for g in range(NG):
                    y0 = (g // XH) * BY
                    pt = pspool.tile([C, GW, 512], f32, tag="pt")
                    ot = opool.tile([C, GW, NWIN], bf16, tag="ot")
                    for k in range(GW):
                        j = (g % XH) * GW + k
                        blk = g * GW + k
                        lhsT = ft[:, blk, :]
                        rhs = qx[:, j, y0:y0 + WY, :]
                        if mm_dtype != "float32":
                            lhsT = lhsT.bitcast(mmdt)
                            rhs = rhs.bitcast(mmdt)
                        nc.tensor.matmul(
                            pt[:, k, 0:NWIN], lhsT, rhs, start=True, stop=True
                        )
                    src = pt[:, :, 0:NWIN]
                    if g % 2 == 0:
                        nc.scalar.mul(ot[:, :, :], src, gain_sb[:, 0:1])
                    else:
                        nc.vector.tensor_scalar_mul(ot[:, :, :], src, gain_sb[:, 0:1])
                    nc.sync.dma_start(out=o_dram[b, yb, g, :, :, :], in_=ot[:, :, :])

    nc.compile()
    return nc


def _get_nc():
    mm_dtype = os.environ.get("KERNEL_MM_DTYPE", "float32")
    if mm_dtype not in _CACHE:
        _CACHE[mm_dtype] = _build(mm_dtype)
    return _CACHE[mm_dtype]


def pack_f(f: np.ndarray) -> np.ndarray:
    """[Bany, C, H, W] f32 -> [Bany, C, NYB, NBLK, BY*BX] block-contiguous."""
    n = f.shape[0]
    v = f.reshape(n, C, NYB, YBLK // BY, BY, XH, GW, BX)
    v = v.transpose(0, 1, 2, 3, 5, 6, 4, 7)   # b,c,yb,y0i,xh,k,ys,xs
    return np.ascontiguousarray(v.reshape(n, C, NYB, NBLK, BY * BX))


def _extract(O: np.ndarray) -> np.ndarray:
    """O: [B, NYB, NG, C(part), GW, NWIN] bf16 -> scores [B, 81, H, W] f32."""
    Of = np.ascontiguousarray(O.astype(np.float32))
    V = Of.reshape(B, NYB, NG // XH, XH, BY, BX, GW, WY, WX)
    sb, syb, sy0, sxh, sys, sxs, sk, swy, swx = V.strides
    T = np.lib.stride_tricks.as_strided(
        V,
        shape=(B, ND, ND, NYB, NG // XH, BY, XH, GW, BX),
        strides=(sb, swy, swx, syb, sy0, sys + swy, sxh, sk, sxs + swx),
    )
    return np.ascontiguousarray(T.reshape(B, ND * ND, H, W))


def make_in_maps(f: np.ndarray, q: np.ndarray, gain: float):
    gain_arr = np.full((C, 1), gain, np.float32)
    fp = pack_f(f)
    return [
        {"f": fp[BLOC * c:BLOC * (c + 1)], "q": q[BLOC * c:BLOC * (c + 1)],
         "gain": gain_arr}
        for c in range(NCORES)
    ]


def kernel(**inputs) -> np.ndarray:
    global LAST_RESULTS
    from concourse.bass_utils import run_bass_kernel_spmd

    f = np.ascontiguousarray(np.asarray(inputs["reference_feat"], dtype=np.float32))
    q = np.ascontiguousarray(np.asarray(inputs["query_feat"], dtype=np.float32))
    gain = float(np.asarray(inputs["init_gain"]).reshape(-1)[0])

    nc = _get_nc()
    in_maps = make_in_maps(f, q, gain)
    res = run_bass_kernel_spmd(nc, in_maps, core_ids=list(range(NCORES)))
    LAST_RESULTS = res

    O = np.stack([res.results[c]["out"] for c in range(NCORES)])
    O = O.reshape(B, NYB, NG, C, GW, NWIN)
    return _extract(O)



# revision 2
# speedup vs baseline: 2.6100x; 2.6100x over previous
"""LocalGOCor (PWC-Net local correlation, radius 4) on 8 Trainium2 NeuronCores.

scores[b, d, y, x] = sum_c (gain * f[b,c,y,x]) * q_zeropad[b, c, y+dy, x+dx]
for d = dy*9+dx, dy/dx in [0,9)  (displacement dy-4, dx-4).

Strategy (data-parallel over batch, 2 samples per core):
  - Image tiled into 8y x 16x pixel blocks (M=128 = PSUM partitions).
  - Per block one TensorE matmul: lhsT = F[c, block 128 pixels] (K=C=128),
    rhs = Q window [c, 16y x 24x] read straight out of the row tile with a
    2D strided AP (rows pitch W).  PSUM[p=(ys,xs), (wy,wx)] holds all
    pairwise dots; the 81 useful displacement values per pixel live at
    (wy,wx) = (ys+dy, xs+dx).
  - Inputs are downcast to bf16 on the host (gain folded into F), matmuls
    run in bf16: half the HBM traffic, 4x TensorE rate.  x-edge panels use
    narrowed matmuls (20-wide) plus small zero-memsets on the output tile,
    so the Q row DMA stays fully contiguous (no x padding pass).
  - ACT/DVE copy PSUM->SBUF into one [128, NG*GW*384] bf16 tile; a single
    ~1.6 MB DMA per (sample, row-chunk) writes it out.  The band
    ("diagonal") extraction is a zero-copy numpy as_strided shear on the
    host during unsharding.
"""

import numpy as np

B, C, H, W = 16, 128, 128, 128
R = 4
ND = 2 * R + 1            # 9 displacements per axis
NCORES = 8
BLOC = B // NCORES        # 2 samples per core
BY, BX = 8, 16            # pixels per block -> M = 128
WY, WX = BY + 2 * R, BX + 2 * R   # 16, 24 query window
NWIN = WY * WX            # 384
YBLK = 32                 # image rows per chunk
NYB = H // YBLK           # 4
QROWS = YBLK + 2 * R      # 40
NPAN = W // BX            # 8 x-panels
GW = 4                    # blocks per PSUM tile (4 banks)
XH = NPAN // GW           # 2 x-halves
NG = (YBLK // BY) * XH    # 8 groups per (b, yblk)
PWX = 32                  # PSUM row pitch (16*32 = 512 = one bank per block)

_CACHE = {}


def _build():
    import concourse.bacc as bacc
    import concourse.tile as tile
    import concourse.mybir as mybir
    from contextlib import ExitStack

    nc = bacc.Bacc(
        "TRN2",
        target_bir_lowering=False,
        debug=False,
        enable_asserts=False,
        num_devices=NCORES,
    )
    f32 = mybir.dt.float32
    bf16 = mybir.dt.bfloat16

    # f is host-packed (and pre-scaled by gain): [BLOC, C, NYB, NBLK, BY*BX]
    f_dram = nc.dram_tensor("f", [BLOC, C, NYB, NG * GW, BY * BX], bf16,
                            kind="ExternalInput").ap()
    q_dram = nc.dram_tensor("q", [BLOC, C, H, W], bf16, kind="ExternalInput").ap()
    o_dram = nc.dram_tensor(
        "out", [BLOC, NYB, C, NG, GW, NWIN], bf16, kind="ExternalOutput"
    ).ap()

    with tile.TileContext(nc) as tc, ExitStack() as ctx:
        qpool = ctx.enter_context(tc.tile_pool(name="qpool", bufs=2))
        fpool = ctx.enter_context(tc.tile_pool(name="fpool", bufs=2))
        opool = ctx.enter_context(tc.tile_pool(name="opool", bufs=2))
        pspool = ctx.enter_context(tc.tile_pool(name="pspool", bufs=2, space="PSUM"))

        for b in range(BLOC):
            for yb in range(NYB):
                ql = qpool.tile([C, QROWS, W], bf16, tag="ql")
                ft = fpool.tile([C, NG * GW, BY * BX], bf16, tag="ft")
                ot = opool.tile([C, NG, GW, WY, WX], bf16, tag="ot")

                nc.sync.dma_start(out=ft[:, :, :], in_=f_dram[b, :, yb, :, :])

                # ql row r <-> padded row yb*YBLK + r <-> real row yb*YBLK+r-4
                r_lo = yb * YBLK - R
                r_hi = yb * YBLK + YBLK + R
                lo_clip, hi_clip = max(r_lo, 0), min(r_hi, H)
                t_lo = lo_clip - r_lo
                t_hi = t_lo + (hi_clip - lo_clip)
                if t_lo > 0:
                    nc.vector.memset(ql[:, 0:t_lo, :], 0.0)
                if t_hi < QROWS:
                    nc.vector.memset(ql[:, t_hi:QROWS, :], 0.0)
                nc.sync.dma_start(
                    out=ql[:, t_lo:t_hi, :],
                    in_=q_dram[b, :, lo_clip:hi_clip, :],
                )

                for g in range(NG):
                    y0 = (g // XH) * BY
                    pt = pspool.tile([C, GW, WY, PWX], f32, tag="pt")
                    for k in range(GW):
                        j = (g % XH) * GW + k
                        blk = g * GW + k
                        # window cols [16j-4, 16j+20) clipped to the image
                        c_lo = 16 * j - R
                        cl, ch = max(c_lo, 0), min(c_lo + WX, W)
                        p_lo = cl - c_lo
                        nc.tensor.matmul(
                            pt[:, k, :, p_lo:p_lo + (ch - cl)],
                            ft[:, blk, :],
                            ql[:, y0:y0 + WY, cl:ch],
                            start=True, stop=True,
                        )
                    src = pt[:, :, :, 0:WX]
                    if g % 2 == 0:
                        nc.scalar.copy(ot[:, g, :, :, :], src)
                    else:
                        nc.vector.tensor_copy(ot[:, g, :, :, :], src)
                    # zero the out-of-image window columns the narrowed
                    # edge matmuls left unwritten (PSUM garbage there)
                    if g % XH == 0:
                        nc.vector.memset(ot[:, g, 0, :, 0:R], 0.0)
                    else:
                        nc.vector.memset(ot[:, g, GW - 1, :, WX - R:WX], 0.0)

                nc.sync.dma_start(out=o_dram[b, yb, :, :, :, :],
                                  in_=ot[:, :, :, :, :])

    nc.compile()
    return nc


def _get_nc():
    if "nc" not in _CACHE:
        _CACHE["nc"] = _build()
    return _CACHE["nc"]


def pack_f(f: np.ndarray, gain: float) -> np.ndarray:
    """[Bany, C, H, W] f32 -> gain-scaled bf16 [Bany, C, NYB, NBLK, BY*BX]
    block-contiguous (block order: (y0i, xh, k) = as emitted on device)."""
    import ml_dtypes
    n = f.shape[0]
    v = (f * np.float32(gain)).astype(ml_dtypes.bfloat16)
    v = v.reshape(n, C, NYB, YBLK // BY, BY, XH, GW, BX)
    v = v.transpose(0, 1, 2, 3, 5, 6, 4, 7)   # b,c,yb,y0i,xh,k,ys,xs
    return np.ascontiguousarray(v.reshape(n, C, NYB, NG * GW, BY * BX))


def _extract(O: np.ndarray) -> np.ndarray:
    """O: [B, NYB, C(part), NG, GW, NWIN] bf16 -> scores [B, 81, H, W] f32."""
    Of = np.ascontiguousarray(O.astype(np.float32))
    # part p = (ys, xs); g = (gy, gx); win = (wy, wx)
    V = Of.reshape(B, NYB, BY, BX, NG // XH, XH, GW, WY, WX)
    sb, syb, sys, sxs, sgy, sgx, sk, swy, swx = V.strides
    T = np.lib.stride_tricks.as_strided(
        V,
        shape=(B, ND, ND, NYB, NG // XH, BY, XH, GW, BX),
        strides=(sb, swy, swx, syb, sgy, sys + swy, sgx, sk, sxs + swx),
    )
    return np.ascontiguousarray(T.reshape(B, ND * ND, H, W))


def make_in_maps(f: np.ndarray, q: np.ndarray, gain: float):
    import ml_dtypes
    fp = pack_f(f, gain)
    qb = q.astype(ml_dtypes.bfloat16)
    return [
        {"f": fp[BLOC * c:BLOC * (c + 1)], "q": qb[BLOC * c:BLOC * (c + 1)]}
        for c in range(NCORES)
    ]


def kernel(**inputs) -> np.ndarray:
    from concourse.bass_utils import run_bass_kernel_spmd

    f = np.ascontiguousarray(np.asarray(inputs["reference_feat"], dtype=np.float32))
    q = np.ascontiguousarray(np.asarray(inputs["query_feat"], dtype=np.float32))
    gain = float(np.asarray(inputs["init_gain"]).reshape(-1)[0])

    nc = _get_nc()
    in_maps = make_in_maps(f, q, gain)
    res = run_bass_kernel_spmd(nc, in_maps, core_ids=list(range(NCORES)))

    O = np.stack([res.results[c]["out"] for c in range(NCORES)])
    O = O.reshape(B, NYB, C, NG, GW, NWIN)
    return _extract(O)
